# revision 26
# baseline (speedup 1.0000x reference)
"""CRF autoencoder loss on 8 TRN2 NeuronCores.

Math: per sequence b,
    la[b] = logsumexp over label paths of (start + sum_t e_t + transitions) + end
    lb[b] = same with emissions e_t + d_t   (d = feature_table[words])
    loss  = sum_b (la - lb)

Strategy (data-parallel over batch, 64 seqs/core):
 - Probability domain: la's log-space scan becomes A_new = em ⊙ (E^T A)
   with E = exp(transitions), em = exp(e - gamma) precomputed host-side
   (gamma keeps magnitudes ~O(1); the scale cancels between la and lb up
   to a closed-form constant added back at the end).  exp(start)/exp(end)
   are folded into the t=0 / t=255 emission columns, so chains start
   directly from a DMA'd emission slice.
 - Bidirectional with an engine-matched asymmetric seam: forward chains
   run on DVE (fused PSUM-multiply, ~425ns/step for 2x64-col chains) and
   cover t=0..SEAM; backward chains run on ACT (PSUM->SBUF copy) + Pool
   (SBUF multiply) at ~615ns/step and cover t=255..SEAM+1.  The seam is
   placed so both sides finish together, which beats the symmetric
   DVE-only split (DVE alone saturates at ~516ns/round for 256 cols).
 - Emissions stream in over two DMA queues (SP: forward half ascending,
   ACT: backward half descending) in chain consumption order.
"""

import numpy as np
import ml_dtypes

import concourse.bacc as bacc
import concourse.mybir as mybir
import concourse.tile as tile
from concourse.bass_utils import run_bass_kernel_spmd

BF16 = mybir.dt.bfloat16
F32 = mybir.dt.float32
NPBF = ml_dtypes.bfloat16
COPY = mybir.ActivationFunctionType.Copy

B, S, L, V = 512, 256, 128, 32000
NCORES = 8
BC = B // NCORES           # 64 sequences per core
HW = 64                    # columns per sub-chain (alpha or beta)
GAMMA_A = float(np.log(128.0) + 1.0)   # per-step rescale for the alpha chain
DELTA = 0.5                            # gamma_beta - gamma_alpha
CORRECTION = -float(B) * S * DELTA     # -65536

SEAM = 151                 # forward chains cover t=0..SEAM (DVE side)

# DMA chunk boundaries (time steps): forward stream ascending on SP,
# backward stream descending on ACT
FRONT_T = [0, 3, 10, 30, 70, 110, SEAM + 1]
BACK_T = [256, 252, 242, 222, 192, 162, SEAM + 1]

_built = None
last_result = None


def _build():
    nc = bacc.Bacc("TRN2")
    # em layout: col = t*128 + q*64 + s  (q=0 alpha, q=1 beta)
    em_p = nc.declare_dram_parameter("em", [L, S * 2 * BC], BF16, isOutput=False)
    tr_p = nc.declare_dram_parameter("tr", [L, L], BF16, isOutput=False)
    trt_p = nc.declare_dram_parameter("trt", [L, L], BF16, isOutput=False)
    out_p = nc.declare_dram_parameter("out", [2 * BC, 1], F32, isOutput=True)

    W = 2 * BC

    with tile.TileContext(nc) as tc:
        with tc.tile_pool(name="const", bufs=1) as cp, \
             tc.tile_pool(name="state", bufs=3) as sp, \
             tc.tile_pool(name="fin", bufs=1) as fp, \
             tc.tile_pool(name="ps", bufs=2, space="PSUM") as pp:

            E = cp.tile([L, L], BF16, tag="E")
            nc.gpsimd.dma_start(E[:], tr_p[:])
            Et = cp.tile([L, L], BF16, tag="Et")
            nc.gpsimd.dma_start(Et[:], trt_p[:])

            ones = cp.tile([L, 1], BF16, tag="ones")
            nc.vector.memset(ones[:], 1.0)

            em = cp.tile([L, S * W], BF16, tag="em")
            for t0, t1 in zip(FRONT_T[:-1], FRONT_T[1:]):
                nc.sync.dma_start(em[:, t0 * W:t1 * W], em_p[:, t0 * W:t1 * W])
            for t1, t0 in zip(BACK_T[:-1], BACK_T[1:]):
                nc.scalar.dma_start(em[:, t0 * W:t1 * W], em_p[:, t0 * W:t1 * W])

            def em_q(t, q):
                c0 = t * W + q * HW
                return em[:, c0:c0 + HW]

            # forward sub-chains (alpha, beta) on DVE; round 1 rhs is the
            # folded t=0 emission slice
            fst = []
            for q in range(2):
                ps = pp.tile([L, HW], F32, tag=f"pf{q}")
                nc.tensor.matmul(ps[:], E[:], em_q(0, q), start=True, stop=True)
                st = sp.tile([L, HW], BF16, tag=f"fs{q}")
                nc.vector.tensor_mul(st[:], ps[:], em_q(1, q))
                fst.append(st)

            # backward sub-chains on ACT copy + Pool multiply
            bst = []
            for q in range(2):
                ps = pp.tile([L, HW], F32, tag=f"pb{q}")
                nc.tensor.matmul(ps[:], Et[:], em_q(S - 1, q), start=True, stop=True)
                cpy = sp.tile([L, HW], BF16, tag=f"bc{q}")
                nc.scalar.activation(cpy[:], ps[:], COPY)
                st = sp.tile([L, HW], BF16, tag=f"bs{q}")
                nc.gpsimd.tensor_mul(st[:], cpy[:], em_q(S - 2, q))
                bst.append(st)

            nf = SEAM - 1                  # forward rounds after round 1
            nb = S - 3 - SEAM              # backward rounds after round 1
            for r in range(max(nf, nb)):
                if r < nf:
                    t = 2 + r              # forward time 2..SEAM
                    for q in range(2):
                        ps = pp.tile([L, HW], F32, tag=f"pf{q}")
                        nc.tensor.matmul(ps[:], E[:], fst[q][:],
                                         start=True, stop=True)
                        st = sp.tile([L, HW], BF16, tag=f"fs{q}")
                        nc.vector.tensor_mul(st[:], ps[:], em_q(t, q))
                        fst[q] = st
                if r < nb:
                    t = S - 3 - r          # backward time 253..SEAM+1
                    for q in range(2):
                        ps = pp.tile([L, HW], F32, tag=f"pb{q}")
                        nc.tensor.matmul(ps[:], Et[:], bst[q][:],
                                         start=True, stop=True)
                        cpy = sp.tile([L, HW], BF16, tag=f"bc{q}")
                        nc.scalar.activation(cpy[:], ps[:], COPY)
                        st = sp.tile([L, HW], BF16, tag=f"bs{q}")
                        nc.gpsimd.tensor_mul(st[:], cpy[:], em_q(t, q))
                        bst[q] = st

            # seam: l[c] = sum_j F_SEAM[j,c] * (E @ G_{SEAM+1})[j,c]; the
            # final log/subtract/sum over columns happens on host
            prod = fp.tile([L, W], BF16)
            for q in range(2):
                psS = pp.tile([L, HW], F32, tag=f"pb{q}")
                nc.tensor.matmul(psS[:], Et[:], bst[q][:], start=True, stop=True)
                nc.vector.tensor_mul(prod[:, q * HW:(q + 1) * HW],
                                     psS[:], fst[q][:])
            pssum = pp.tile([W, 1], F32, tag="pf0")
            nc.tensor.matmul(pssum[:], prod[:], ones[:], start=True, stop=True)
            lsum = fp.tile([W, 1], F32)
            nc.vector.tensor_scalar_mul(lsum[:], pssum[:], 1.0)
            nc.sync.dma_start(out_p[:], lsum[:])

    nc.compile()
    return nc


def _get_nc():
    global _built
    if _built is None:
        _built = _build()
    return _built


def kernel(words, encoder_emits, mask, feature_table, start, transitions, end):
    global last_result
    words = np.asarray(words)
    encoder_emits = np.asarray(encoder_emits, dtype=np.float32)
    feature_table = np.asarray(feature_table, dtype=np.float32)
    start = np.asarray(start, dtype=np.float32)
    transitions = np.asarray(transitions, dtype=np.float32)
    end = np.asarray(end, dtype=np.float32)
    assert words.shape == (B, S) and encoder_emits.shape == (B, S, L)

    d = feature_table[words]                       # [B, S, L]
    ea = np.exp(encoder_emits - GAMMA_A)
    eb = ea * np.exp(d - DELTA)
    st_f = np.exp(start)[None, :]
    en_f = np.exp(end)[None, :]
    ea[:, 0, :] *= st_f
    eb[:, 0, :] *= st_f
    ea[:, S - 1, :] *= en_f
    eb[:, S - 1, :] *= en_f

    tr = np.ascontiguousarray(np.exp(transitions), dtype=NPBF)
    trt = np.ascontiguousarray(np.exp(transitions).T, dtype=NPBF)

    in_maps = []
    for c in range(NCORES):
        sl = slice(c * BC, (c + 1) * BC)
        # em[l, t*128 + q*64 + s]: stack alpha/beta per step
        both = np.stack([ea[sl], eb[sl]], axis=2)   # [BC, S, 2, L]
        em = np.ascontiguousarray(
            both.astype(NPBF).transpose(3, 1, 2, 0)).reshape(L, S * 2 * BC)
        in_maps.append({"em": em, "tr": tr, "trt": trt})

    nc = _get_nc()
    res = run_bass_kernel_spmd(nc, in_maps, core_ids=list(range(NCORES)))
    last_result = res
    total = 0.0
    for r in res.results:
        z = np.log(np.asarray(r["out"], dtype=np.float64).reshape(2 * BC))
        total += float(np.sum(z[:BC] - z[BC:]))
    return np.array(total + CORRECTION, dtype=np.float32)


# revision 27
# speedup vs baseline: 1.1081x; 1.1081x over previous
"""CRF autoencoder loss on 8 TRN2 NeuronCores.

Math: per sequence b,
    la[b] = logsumexp over label paths of (start + sum_t e_t + transitions) + end
    lb[b] = same with emissions e_t + d_t   (d = feature_table[words])
    loss  = sum_b (la - lb)

Strategy (data-parallel over batch, 64 seqs/core):
 - Probability domain: la's log-space scan becomes A_new = em ⊙ (E^T A)
   with E = exp(transitions), em = exp(e - gamma) precomputed host-side
   (gamma keeps magnitudes ~O(1); the scale cancels between la and lb up
   to a closed-form constant added back at the end).  exp(start)/exp(end)
   are folded into the t=0 / t=255 emission columns, so chains start
   directly from a DMA'd emission slice.
 - Bidirectional: a forward chain covers t=0..127 and a backward chain
   t=255..128, joined by a dot product at the seam; serial depth 127.
 - Each chain carries [128 labels, 64 alpha cols | 64 beta cols]; each
   round is one [128,128]@[128,128] PE matmul into PSUM plus one DVE
   tensor_mul applying the emission.  Two interleaved chains keep DVE
   ~100% busy at ~516ns/round, which is the legal throughput/latency
   optimum on TRN2 (GPSIMD may not read PSUM; a 3rd engine hop adds more
   latency than it saves).
 - All emission tensors stream in over two DMA queues (SP: forward half
   ascending, ACT: backward half descending) in consumption order, so
   the chains never stall on supply.
"""

import numpy as np
import ml_dtypes

import concourse.bacc as bacc
import concourse.mybir as mybir
import concourse.tile as tile
from concourse.bass_utils import run_bass_kernel_spmd

BF16 = mybir.dt.bfloat16
F32 = mybir.dt.float32
NPBF = ml_dtypes.bfloat16
LN = mybir.ActivationFunctionType.Ln

B, S, L, V = 512, 256, 128, 32000
NCORES = 8
BC = B // NCORES           # 64 sequences per core
GAMMA_A = float(np.log(128.0) + 1.0)   # per-step rescale for the alpha chain
DELTA = 0.5                            # gamma_beta - gamma_alpha
# Each of the S emission factors is scaled by exp(-gamma); summed over all
# sequences: loss_true = loss_dev + B*S*(gamma_a - gamma_b).
CORRECTION = -float(B) * S * DELTA     # -65536

# time-chunk boundaries for the two DMA streams (cols = t*128); geometric
# ramp so each chunk lands just before the chains consume it
FRONT_T = [0, 3, 8, 20, 44, 84, 128]
BACK_T = [256, 253, 248, 236, 212, 172, 128]

_built = None
last_result = None


def _build():
    nc = bacc.Bacc("TRN2")
    # em layout: col = t*128 + q*64 + s  (q=0 alpha, q=1 beta)
    em_p = nc.declare_dram_parameter("em", [L, S * 2 * BC], BF16, isOutput=False)
    tr_p = nc.declare_dram_parameter("tr", [L, L], BF16, isOutput=False)
    trt_p = nc.declare_dram_parameter("trt", [L, L], BF16, isOutput=False)
    out_p = nc.declare_dram_parameter("out", [2 * BC, 1], F32, isOutput=True)

    W = 2 * BC  # 128 state columns per chain

    with tile.TileContext(nc) as tc:
        with tc.tile_pool(name="const", bufs=1) as cp, \
             tc.tile_pool(name="state", bufs=3) as sp, \
             tc.tile_pool(name="fin", bufs=1) as fp, \
             tc.tile_pool(name="ps", bufs=2, space="PSUM") as pp:

            # E/Et on the gpsimd-issued DMA queue so the SP/ACT emission
            # streams start immediately
            E = cp.tile([L, L], BF16, tag="E")
            nc.gpsimd.dma_start(E[:], tr_p[:])
            Et = cp.tile([L, L], BF16, tag="Et")
            nc.gpsimd.dma_start(Et[:], trt_p[:])

            ones = cp.tile([L, 1], BF16, tag="ones")
            nc.vector.memset(ones[:], 1.0)

            em = cp.tile([L, S * W], BF16, tag="em")
            for t0, t1 in zip(FRONT_T[:-1], FRONT_T[1:]):
                nc.sync.dma_start(em[:, t0 * W:t1 * W], em_p[:, t0 * W:t1 * W])
            for t1, t0 in zip(BACK_T[:-1], BACK_T[1:]):
                nc.scalar.dma_start(em[:, t0 * W:t1 * W], em_p[:, t0 * W:t1 * W])

            def em_t(t):
                return em[:, t * W:(t + 1) * W]

            # round 1: rhs is the folded t=0 / t=255 emission slice itself
            psf = pp.tile([L, W], F32, tag="psf")
            nc.tensor.matmul(psf[:], E[:], em_t(0), start=True, stop=True)
            fstate = sp.tile([L, W], BF16, tag="fs")
            nc.vector.tensor_mul(fstate[:], psf[:], em_t(1))

            psb = pp.tile([L, W], F32, tag="psb")
            nc.tensor.matmul(psb[:], Et[:], em_t(S - 1), start=True, stop=True)
            bstate = sp.tile([L, W], BF16, tag="bs")
            nc.vector.tensor_mul(bstate[:], psb[:], em_t(S - 2))

            for k in range(2, S // 2):
                psf = pp.tile([L, W], F32, tag="psf")
                nc.tensor.matmul(psf[:], E[:], fstate[:], start=True, stop=True)
                nf = sp.tile([L, W], BF16, tag="fs")
                nc.vector.tensor_mul(nf[:], psf[:], em_t(k))
                fstate = nf

                psb = pp.tile([L, W], F32, tag="psb")
                nc.tensor.matmul(psb[:], Et[:], bstate[:], start=True, stop=True)
                nb = sp.tile([L, W], BF16, tag="bs")
                nc.vector.tensor_mul(nb[:], psb[:], em_t(S - 1 - k))
                bstate = nb

            # seam: l[c] = sum_j fwd127[j,c] * (E @ bwd128)[j,c]; the final
            # log/subtract/sum over the 128 per-column sums happens on host
            psfin = pp.tile([L, W], F32, tag="psb")
            nc.tensor.matmul(psfin[:], Et[:], bstate[:], start=True, stop=True)
            prod = fp.tile([L, W], BF16)
            nc.vector.tensor_mul(prod[:], psfin[:], fstate[:])
            # colsum with prod as stationary -> [128,1] output (free size 1:
            # near-zero PE + copy cost in the tail)
            pssum = pp.tile([W, 1], F32, tag="pssum")
            nc.tensor.matmul(pssum[:], prod[:], ones[:], start=True, stop=True)
            lsum = fp.tile([W, 1], F32)
            nc.vector.tensor_scalar_mul(lsum[:], pssum[:], 1.0)
            nc.sync.dma_start(out_p[:], lsum[:])

    nc.compile()
    return nc


def _get_nc():
    global _built
    if _built is None:
        _built = _build()
    return _built


def kernel(words, encoder_emits, mask, feature_table, start, transitions, end):
    global last_result
    words = np.asarray(words)
    encoder_emits = np.asarray(encoder_emits, dtype=np.float32)
    feature_table = np.asarray(feature_table, dtype=np.float32)
    start = np.asarray(start, dtype=np.float32)
    transitions = np.asarray(transitions, dtype=np.float32)
    end = np.asarray(end, dtype=np.float32)
    assert words.shape == (B, S) and encoder_emits.shape == (B, S, L)

    d = feature_table[words]                       # [B, S, L]
    ea = np.exp(encoder_emits - GAMMA_A)
    eb = ea * np.exp(d - DELTA)
    st_f = np.exp(start)[None, :]
    en_f = np.exp(end)[None, :]
    ea[:, 0, :] *= st_f
    eb[:, 0, :] *= st_f
    ea[:, S - 1, :] *= en_f
    eb[:, S - 1, :] *= en_f

    tr = np.ascontiguousarray(np.exp(transitions), dtype=NPBF)
    trt = np.ascontiguousarray(np.exp(transitions).T, dtype=NPBF)

    in_maps = []
    for c in range(NCORES):
        sl = slice(c * BC, (c + 1) * BC)
        # em[l, t*128 + q*64 + s]: stack alpha/beta per step
        both = np.stack([ea[sl], eb[sl]], axis=2)   # [BC, S, 2, L]
        em = np.ascontiguousarray(
            both.astype(NPBF).transpose(3, 1, 2, 0)).reshape(L, S * 2 * BC)
        in_maps.append({"em": em, "tr": tr, "trt": trt})

    nc = _get_nc()
    res = run_bass_kernel_spmd(nc, in_maps, core_ids=list(range(NCORES)))
    last_result = res
    total = 0.0
    for r in res.results:
        z = np.log(np.asarray(r["out"], dtype=np.float64).reshape(2 * BC))
        total += float(np.sum(z[:BC] - z[BC:]))
    return np.array(total + CORRECTION, dtype=np.float32)


# revision 28
# speedup vs baseline: 1.1111x; 1.0027x over previous
"""CRF autoencoder loss on 8 TRN2 NeuronCores.

Math: per sequence b,
    la[b] = logsumexp over label paths of (start + sum_t e_t + transitions) + end
    lb[b] = same with emissions e_t + d_t   (d = feature_table[words])
    loss  = sum_b (la - lb)

Strategy (data-parallel over batch, 64 seqs/core):
 - Probability domain: la's log-space scan becomes A_new = em ⊙ (E^T A)
   with E = exp(transitions), em = exp(e - gamma) precomputed host-side
   (gamma keeps magnitudes ~O(1); the scale cancels between la and lb up
   to a closed-form constant added back at the end).  exp(start)/exp(end)
   are folded into the t=0 / t=255 emission columns, so chains start
   directly from a DMA'd emission slice.
 - Bidirectional: a forward chain covers t=0..127 and a backward chain
   t=255..128, joined by a dot product at the seam; serial depth 127.
 - Each chain carries [128 labels, 64 alpha cols | 64 beta cols]; each
   round is one [128,128]@[128,128] PE matmul into PSUM plus one DVE
   tensor_mul applying the emission.  Two interleaved chains keep DVE
   ~100% busy at ~516ns/round, which is the legal throughput/latency
   optimum on TRN2 (GPSIMD may not read PSUM; a 3rd engine hop adds more
   latency than it saves).
 - All emission tensors stream in over two DMA queues (SP: forward half
   ascending, ACT: backward half descending) in consumption order, so
   the chains never stall on supply.
"""

import numpy as np
import ml_dtypes

import concourse.bacc as bacc
import concourse.mybir as mybir
import concourse.tile as tile
from concourse.bass_utils import run_bass_kernel_spmd

BF16 = mybir.dt.bfloat16
F32 = mybir.dt.float32
NPBF = ml_dtypes.bfloat16
LN = mybir.ActivationFunctionType.Ln

B, S, L, V = 512, 256, 128, 32000
NCORES = 8
BC = B // NCORES           # 64 sequences per core
GAMMA_A = float(np.log(128.0) + 1.0)   # per-step rescale for the alpha chain
DELTA = 0.5                            # gamma_beta - gamma_alpha
# Each of the S emission factors is scaled by exp(-gamma); summed over all
# sequences: loss_true = loss_dev + B*S*(gamma_a - gamma_b).
CORRECTION = -float(B) * S * DELTA     # -65536

# time-chunk boundaries for the two DMA streams (cols = t*128); geometric
# ramp so each chunk lands just before the chains consume it
FRONT_T = [0, 3, 8, 20, 44, 84, 128]
BACK_T = [256, 253, 248, 236, 212, 172, 128]

_built = None
last_result = None


def _build():
    nc = bacc.Bacc("TRN2")
    # em layout: col = t*128 + q*64 + s  (q=0 alpha, q=1 beta)
    em_p = nc.declare_dram_parameter("em", [L, S * 2 * BC], BF16, isOutput=False)
    tr_p = nc.declare_dram_parameter("tr", [L, 2 * L], BF16, isOutput=False)
    out_p = nc.declare_dram_parameter("out", [2 * BC, 1], F32, isOutput=True)

    W = 2 * BC  # 128 state columns per chain

    with tile.TileContext(nc) as tc:
        with tc.tile_pool(name="const", bufs=1) as cp, \
             tc.tile_pool(name="state", bufs=3) as sp, \
             tc.tile_pool(name="fin", bufs=1) as fp, \
             tc.tile_pool(name="ps", bufs=2, space="PSUM") as pp:

            # E|Et fused in one transfer on the gpsimd-issued DMA queue so
            # the SP/ACT emission streams start immediately
            EE = cp.tile([L, 2 * L], BF16, tag="EE")
            nc.gpsimd.dma_start(EE[:], tr_p[:])
            E = EE[:, 0:L]
            Et = EE[:, L:2 * L]

            ones = cp.tile([L, 1], BF16, tag="ones")
            nc.vector.memset(ones[:], 1.0)

            em = cp.tile([L, S * W], BF16, tag="em")
            for t0, t1 in zip(FRONT_T[:-1], FRONT_T[1:]):
                nc.sync.dma_start(em[:, t0 * W:t1 * W], em_p[:, t0 * W:t1 * W])
            for t1, t0 in zip(BACK_T[:-1], BACK_T[1:]):
                nc.scalar.dma_start(em[:, t0 * W:t1 * W], em_p[:, t0 * W:t1 * W])

            def em_t(t):
                return em[:, t * W:(t + 1) * W]

            # round 1: rhs is the folded t=0 / t=255 emission slice itself
            psf = pp.tile([L, W], F32, tag="psf")
            nc.tensor.matmul(psf[:], E, em_t(0), start=True, stop=True)
            fstate = sp.tile([L, W], BF16, tag="fs")
            nc.vector.tensor_mul(fstate[:], psf[:], em_t(1))

            psb = pp.tile([L, W], F32, tag="psb")
            nc.tensor.matmul(psb[:], Et, em_t(S - 1), start=True, stop=True)
            bstate = sp.tile([L, W], BF16, tag="bs")
            nc.vector.tensor_mul(bstate[:], psb[:], em_t(S - 2))

            for k in range(2, S // 2):
                psf = pp.tile([L, W], F32, tag="psf")
                nc.tensor.matmul(psf[:], E, fstate[:], start=True, stop=True)
                nf = sp.tile([L, W], BF16, tag="fs")
                nc.vector.tensor_mul(nf[:], psf[:], em_t(k))
                fstate = nf

                psb = pp.tile([L, W], F32, tag="psb")
                nc.tensor.matmul(psb[:], Et, bstate[:], start=True, stop=True)
                nb = sp.tile([L, W], BF16, tag="bs")
                nc.vector.tensor_mul(nb[:], psb[:], em_t(S - 1 - k))
                bstate = nb

            # seam: l[c] = sum_j fwd127[j,c] * (E @ bwd128)[j,c]; the final
            # log/subtract/sum over the 128 per-column sums happens on host
            psfin = pp.tile([L, W], F32, tag="psb")
            nc.tensor.matmul(psfin[:], Et, bstate[:], start=True, stop=True)
            prod = fp.tile([L, W], BF16)
            nc.vector.tensor_mul(prod[:], psfin[:], fstate[:])
            # colsum with prod as stationary -> [128,1] output (free size 1:
            # near-zero PE + copy cost in the tail)
            pssum = pp.tile([W, 1], F32, tag="pssum")
            nc.tensor.matmul(pssum[:], prod[:], ones[:], start=True, stop=True)
            lsum = fp.tile([W, 1], F32)
            nc.vector.tensor_scalar_mul(lsum[:], pssum[:], 1.0)
            nc.sync.dma_start(out_p[:], lsum[:])

    nc.compile()
    return nc


def _get_nc():
    global _built
    if _built is None:
        _built = _build()
    return _built


def kernel(words, encoder_emits, mask, feature_table, start, transitions, end):
    global last_result
    words = np.asarray(words)
    encoder_emits = np.asarray(encoder_emits, dtype=np.float32)
    feature_table = np.asarray(feature_table, dtype=np.float32)
    start = np.asarray(start, dtype=np.float32)
    transitions = np.asarray(transitions, dtype=np.float32)
    end = np.asarray(end, dtype=np.float32)
    assert words.shape == (B, S) and encoder_emits.shape == (B, S, L)

    d = feature_table[words]                       # [B, S, L]
    ea = np.exp(encoder_emits - GAMMA_A)
    eb = ea * np.exp(d - DELTA)
    st_f = np.exp(start)[None, :]
    en_f = np.exp(end)[None, :]
    ea[:, 0, :] *= st_f
    eb[:, 0, :] *= st_f
    ea[:, S - 1, :] *= en_f
    eb[:, S - 1, :] *= en_f

    trE = np.exp(transitions)
    tr = np.ascontiguousarray(
        np.concatenate([trE, trE.T], axis=1), dtype=NPBF)

    in_maps = []
    for c in range(NCORES):
        sl = slice(c * BC, (c + 1) * BC)
        # em[l, t*128 + q*64 + s]: stack alpha/beta per step
        both = np.stack([ea[sl], eb[sl]], axis=2)   # [BC, S, 2, L]
        em = np.ascontiguousarray(
            both.astype(NPBF).transpose(3, 1, 2, 0)).reshape(L, S * 2 * BC)
        in_maps.append({"em": em, "tr": tr})

    nc = _get_nc()
    res = run_bass_kernel_spmd(nc, in_maps, core_ids=list(range(NCORES)))
    last_result = res
    total = 0.0
    for r in res.results:
        z = np.log(np.asarray(r["out"], dtype=np.float64).reshape(2 * BC))
        total += float(np.sum(z[:BC] - z[BC:]))
    return np.array(total + CORRECTION, dtype=np.float32)
